# revision 3
# baseline (speedup 1.0000x reference)
"""Distributed embedding lookup (bag gather + masked mean) on 8 Trainium2 cores.

Strategy: data-parallel over the batch. Each core holds a full replica of the
embedding table in its HBM and processes 512 of the 4096 batch rows:
  - host marshals keys/mask into a [128 partitions, 104 tiles * 10 nnz] layout
  - device remaps masked-out keys to a sentinel row (appended zero row in the
    table) so the gather itself zeroes invalid entries
  - one indirect DMA per 1024 slots gathers 10240 embedding rows into SBUF
  - DVE reduces each slot's 10 rows (tree add) and scales by 1/max(count,1)
No collectives needed: replication beats the key%8 model-parallel split since
the 256 MB table fits per-core and reduce-scatter traffic is avoided.
"""

import numpy as np

# Problem constants (hardcoded per harness contract).
B, S, N, E, V = 4096, 26, 10, 64, 1_000_000
NCORES = 8
BL = B // NCORES              # 512 batch rows per core
SL = BL * S                   # 13312 slots per core
P = 128                       # SBUF partitions
NT = SL // P                  # 104 tiles of 128 slots
GT = 8                        # tiles per gather super-tile
NSUP = NT // GT               # 13 super-tiles
KPS = GT * N                  # 80 keys per partition per super-tile
VPAD = V + 8                  # table padded with 8 zero rows; sentinel = V

_STATE = {}


def _build_nc():
    import concourse.bacc as bacc
    import concourse.mybir as mybir
    import concourse.tile as tile

    nc = bacc.Bacc("TRN2", target_bir_lowering=False, debug=False,
                   num_devices=NCORES)
    keys_t = nc.declare_dram_parameter("keys_t", [P, NT * N], mybir.dt.int32,
                                       isOutput=False)
    mask_t = nc.declare_dram_parameter("mask_t", [P, NT * N], mybir.dt.int32,
                                       isOutput=False)
    table_t = nc.declare_dram_parameter("table_t", [VPAD, E], mybir.dt.float32,
                                        isOutput=False)
    out_t = nc.declare_dram_parameter("out_t", [P, NT * E], mybir.dt.float32,
                                      isOutput=True)

    import concourse.bass as bass
    f32 = mybir.dt.float32
    i32 = mybir.dt.int32

    with tile.TileContext(nc) as tc:
        with (
            tc.tile_pool(name="persist", bufs=1) as persist,
            tc.tile_pool(name="gather", bufs=3) as gpool,
            tc.tile_pool(name="tmp", bufs=4) as tpool,
            tc.tile_pool(name="outp", bufs=3) as opool,
        ):
            keys_sb = persist.tile([P, NT * N], i32)
            mask_sb = persist.tile([P, NT * N], i32)
            adj_sb = persist.tile([P, NT * N], i32)
            counts_i = persist.tile([P, NT], i32)
            counts_f = persist.tile([P, NT], f32)
            recip = persist.tile([P, NT], f32)

            nc.sync.dma_start(out=keys_sb[:], in_=keys_t[:])
            nc.sync.dma_start(out=mask_sb[:], in_=mask_t[:])

            # counts per slot = sum of mask over the 10 nnz positions
            with nc.allow_low_precision(reason="int32 sum of 10 0/1 values"):
                nc.vector.tensor_reduce(
                    out=counts_i[:],
                    in_=mask_sb[:].rearrange("p (t n) -> p t n", n=N),
                    axis=mybir.AxisListType.X,
                    op=mybir.AluOpType.add,
                )
            nc.vector.tensor_copy(out=counts_f[:], in_=counts_i[:])
            nc.vector.tensor_scalar_max(out=counts_f[:], in0=counts_f[:],
                                        scalar1=1.0)
            nc.vector.reciprocal(out=recip[:], in_=counts_f[:])

            # adj = mask ? key : V  (V indexes the appended zero row)
            # adj = (key - V) * mask + V
            nc.vector.tensor_scalar_add(out=adj_sb[:], in0=keys_sb[:],
                                        scalar1=-V)
            nc.vector.tensor_tensor(out=adj_sb[:], in0=adj_sb[:],
                                    in1=mask_sb[:], op=mybir.AluOpType.mult)
            nc.vector.tensor_scalar_add(out=adj_sb[:], in0=adj_sb[:],
                                        scalar1=V)

            for g in range(NSUP):
                gt = gpool.tile([P, GT * N * E], f32)
                # HW indirect DMA consumes ONE index per dest partition row:
                # issue one gather per key column, [P,1] idx -> [P,E] dest.
                for j in range(KPS):
                    nc.gpsimd.indirect_dma_start(
                        out=gt[:, j * E:(j + 1) * E],
                        out_offset=None,
                        in_=table_t[:],
                        in_offset=bass.IndirectOffsetOnAxis(
                            ap=adj_sb[:, g * KPS + j:g * KPS + j + 1], axis=0),
                    )
                osup = opool.tile([P, GT * E], f32)
                for i in range(GT):
                    tt = g * GT + i
                    sl = gt[:, i * N * E:(i + 1) * N * E]
                    t320 = tpool.tile([P, 5 * E], f32)
                    t128 = tpool.tile([P, 2 * E], f32)
                    t64 = tpool.tile([P, E], f32)
                    nc.vector.tensor_add(out=t320[:], in0=sl[:, 0:5 * E],
                                         in1=sl[:, 5 * E:10 * E])
                    nc.vector.tensor_add(out=t128[:], in0=t320[:, 0:2 * E],
                                         in1=t320[:, 2 * E:4 * E])
                    nc.vector.tensor_add(out=t64[:], in0=t128[:, 0:E],
                                         in1=t128[:, E:2 * E])
                    nc.vector.tensor_add(out=t64[:], in0=t64[:],
                                         in1=t320[:, 4 * E:5 * E])
                    nc.vector.tensor_scalar_mul(
                        out=osup[:, i * E:(i + 1) * E], in0=t64[:],
                        scalar1=recip[:, tt:tt + 1])
                nc.sync.dma_start(out=out_t[:, g * GT * E:(g + 1) * GT * E],
                                  in_=osup[:])
    nc.compile()
    return nc


def _make_runner(nc):
    import jax
    import concourse.mybir as mybir
    from concourse import bass2jax
    from jax.sharding import Mesh, PartitionSpec
    from jax.experimental.shard_map import shard_map

    bass2jax.install_neuronx_cc_hook()

    in_names, out_names, out_avals, zero_shapes = [], [], [], []
    partition_name = (nc.partition_id_tensor.name
                      if nc.partition_id_tensor else None)
    for alloc in nc.m.functions[0].allocations:
        if not isinstance(alloc, mybir.MemoryLocationSet):
            continue
        name = alloc.memorylocations[0].name
        if alloc.kind == "ExternalInput":
            if name != partition_name:
                in_names.append(name)
        elif alloc.kind == "ExternalOutput":
            out_names.append(name)
            shape = tuple(alloc.tensor_shape)
            dtype = mybir.dt.np(alloc.dtype)
            out_avals.append(jax.core.ShapedArray(shape, dtype))
            zero_shapes.append((shape, dtype))
    n_params = len(in_names)
    n_outs = len(out_avals)
    all_in_names = list(in_names) + list(out_names)
    if partition_name is not None:
        all_in_names.append(partition_name)
    donate = tuple(range(n_params, n_params + n_outs))

    def _body(*args):
        operands = list(args)
        if partition_name is not None:
            operands.append(bass2jax.partition_id_tensor())
        outs = bass2jax._bass_exec_p.bind(
            *operands,
            out_avals=tuple(out_avals),
            in_names=tuple(all_in_names),
            out_names=tuple(out_names),
            lowering_input_output_aliases=(),
            sim_require_finite=True,
            sim_require_nnan=True,
            nc=nc,
        )
        return tuple(outs)

    devices = jax.devices()[:NCORES]
    mesh = Mesh(np.asarray(devices), ("core",))
    # keys/mask sharded by core; table replicated; donated output sharded
    in_specs = (PartitionSpec("core"), PartitionSpec("core"), PartitionSpec(),
                PartitionSpec("core"))
    out_specs = (PartitionSpec("core"),)
    fn = jax.jit(
        shard_map(_body, mesh=mesh, in_specs=in_specs, out_specs=out_specs,
                  check_rep=False),
        donate_argnums=donate, keep_unused=True,
    )
    return fn, mesh, in_names, out_names, zero_shapes


def _get_state():
    if "fn" not in _STATE:
        nc = _build_nc()
        fn, mesh, in_names, out_names, zero_shapes = _make_runner(nc)
        _STATE.update(nc=nc, fn=fn, mesh=mesh, in_names=in_names,
                      out_names=out_names, zero_shapes=zero_shapes)
    return _STATE


def _marshal_percore(arr_c):
    """[BL, S, N] -> [P, NT*N] partition-major tile layout."""
    return (arr_c.reshape(SL, N).reshape(NT, P, N).transpose(1, 0, 2)
            .reshape(P, NT * N))


def marshal_inputs(keys, mask):
    keys_g = np.empty((NCORES * P, NT * N), np.int32)
    mask_g = np.empty((NCORES * P, NT * N), np.int32)
    for c in range(NCORES):
        sl = slice(c * BL, (c + 1) * BL)
        keys_g[c * P:(c + 1) * P] = _marshal_percore(
            np.ascontiguousarray(keys[sl], dtype=np.int32))
        mask_g[c * P:(c + 1) * P] = _marshal_percore(
            mask[sl].astype(np.int32))
    return keys_g, mask_g


def pad_table(table):
    table_ext = np.zeros((VPAD, E), np.float32)
    table_ext[:V] = table
    return table_ext


def unmarshal_output(out_g):
    """[NCORES*P, NT*E] -> [B, S, E]"""
    out = np.empty((B, S, E), np.float32)
    for c in range(NCORES):
        oc = np.asarray(out_g[c * P:(c + 1) * P])  # [P, NT*E]
        out[c * BL:(c + 1) * BL] = (
            oc.reshape(P, NT, E).transpose(1, 0, 2).reshape(BL, S, E))
    return out


def kernel(keys, mask, table):
    import jax
    from jax.sharding import NamedSharding, PartitionSpec

    st = _get_state()
    keys_g, mask_g = marshal_inputs(np.asarray(keys), np.asarray(mask))

    tkey = id(table)
    if _STATE.get("table_key") != tkey:
        table_ext = pad_table(np.asarray(table, dtype=np.float32))
        _STATE["table_dev"] = jax.device_put(
            table_ext, NamedSharding(st["mesh"], PartitionSpec()))
        _STATE["table_key"] = tkey

    zshape, zdtype = st["zero_shapes"][0]
    zeros_out = np.zeros((NCORES * zshape[0], *zshape[1:]), zdtype)
    outs = st["fn"](keys_g, mask_g, _STATE["table_dev"], zeros_out)
    out_g = np.asarray(jax.block_until_ready(outs[0]))
    return unmarshal_output(out_g)
